# revision 61
# baseline (speedup 1.0000x reference)
"""AudioCrossAttention on 8 Trainium2 NeuronCores.

Sharding: data-parallel over batch (B=2) x tensor-parallel over heads
(16 heads -> 4 heads / 256 dims per core).  Core c handles batch c//4 and
head-group c%4.  Each core computes its 4 heads' attention plus the partial
output projection over its 256-dim slice; partials are summed on the host
(the unshard step) and bo added there.

Everything on device flows in transposed layout ([d, s] / [skv, sq]) so no
transposes are ever needed:
  qT[d,sq]  = WqT.T @ xT          (lhsT=WqT [din,256], rhs=visual.T)
  kT[d,skv] = WkT.T @ xT (+bk +L-RoPE emb, fused into the PSUM eviction)
  v[skv,d]  = xT.T @ WvT  (natural layout, ones column appended per head)
  scoresT[skv,sq] = kT_h.T @ qT_h          per head, K=hd=64
  expT = exp(0.125 * scoresT)              (no max-subtract; scores are O(5))
  [outT; denom] = [v_h | 1].T @ expT       (ones column -> row 64 = denom)
  outT /= denom  (reciprocal -> gpsimd partition_broadcast -> DVE mult)
  finalT[e,sq] += WoT_c.T @ outT           (partial over this core's d-slice)

The scores matmuls have K=hd=64, so heads 2j (SBUF partitions 0-63) and
2j+1 (partitions 64-127) run as concurrent 64x128 row-tiles T0/T8 of the
PE array when their matmuls alternate in the instruction stream (HW
verified 2.0x).  Phase 2 walks head-pair "slots"; each slot's scores+exp
are zippered with the PREVIOUS pair's AV matmuls so the PE keeps working
while ACT (the exp engine, ~2.1us per score-pair) drains.

Matmul operands are fp16 (cast on host): 1 col/cycle at 2.4 GHz on the PE
vs 4x slower fp32 and 2x slower f32r; accumulation stays fp32 in PSUM, the
final projection partials leave the chip in fp32.
"""

import sys

if '/opt/trn_rl_repo' not in sys.path:
    sys.path.insert(0, '/opt/trn_rl_repo')

import numpy as np

B = 2
SQ = 2048
SKV = 2048
DIM = 1024
NUM_HEADS = 16
HEAD_DIM = 64
N_CORES = 8
HPC = 4          # heads per core
DSL = 256        # d_out slice per core
CH = 512         # sq chunk width
NCH = SQ // CH   # 4
KT = DIM // 128  # 8  d_in k-tiles
ST = SKV // 128  # 16 skv tiles
SCALE = HEAD_DIM ** -0.5
AV_SPLIT = True  # row-tiled T0/T8 AV halves (False: full-array M=65 AV)
DVE_EXP_S2 = ()  # skv-tiles whose exp runs on DVE (tested: hurts; keep empty)

_CACHE = {}


def _build():
    import concourse.bacc as bacc
    import concourse.mybir as mybir
    from concourse import tile

    F32 = mybir.dt.float32
    F16 = mybir.dt.float16
    AF = mybir.ActivationFunctionType
    ALU = mybir.AluOpType

    nc = bacc.Bacc("TRN2", target_bir_lowering=False, debug=False,
                   num_devices=N_CORES)

    xq = nc.dram_tensor("xq", [DIM, SQ], F16, kind="ExternalInput")
    xa = nc.dram_tensor("xa", [DIM, SKV], F16, kind="ExternalInput")
    wq = nc.dram_tensor("wq", [DIM, DSL], F16, kind="ExternalInput")
    wk = nc.dram_tensor("wk", [DIM, DSL], F16, kind="ExternalInput")
    wv = nc.dram_tensor("wv", [DIM, DSL], F16, kind="ExternalInput")
    wo = nc.dram_tensor("wo", [DSL, DIM], F16, kind="ExternalInput")
    emb2 = nc.dram_tensor("emb2", [128, SKV], F16, kind="ExternalInput")
    bq2 = nc.dram_tensor("bq2", [128, 2], F32, kind="ExternalInput")
    bk2 = nc.dram_tensor("bk2", [128, 2], F32, kind="ExternalInput")
    bvr = nc.dram_tensor("bvr", [1, DSL], F32, kind="ExternalInput")
    out = nc.dram_tensor("out", [DIM, SQ], F16, kind="ExternalOutput")

    with tile.TileContext(nc) as tc:
        with tc.tile_pool(name="consts", bufs=1) as consts, \
             tc.tile_pool(name="big", bufs=1) as big, \
             tc.tile_pool(name="xqp", bufs=2) as xqp, \
             tc.tile_pool(name="xap", bufs=2) as xap, \
             tc.tile_pool(name="expp", bufs=32) as expp, \
             tc.tile_pool(name="evp", bufs=6) as evp, \
             tc.tile_pool(name="smallp", bufs=3) as smallp, \
             tc.tile_pool(name="ps512", bufs=2, space="PSUM") as ps512, \
             tc.tile_pool(name="ps1024", bufs=2, space="PSUM") as ps1024, \
             tc.tile_pool(name="psav", bufs=2, space="PSUM") as psav:

            # ---- constants (ordered by first use so the leading matmuls
            # aren't stuck behind DMAs of late-phase weights) ----
            wk_sb = consts.tile([128, KT, DSL], F16, tag="wk")
            nc.sync.dma_start(out=wk_sb, in_=wk.rearrange("(kt p) m -> p kt m", p=128))
            # first xa chunk right behind wk (split in halves so k-proj can
            # start after the first 4 k-tiles land)
            xa0 = xap.tile([128, KT, CH], F16, tag="xa", name="xa_c0")
            xa_r = xa.rearrange("(kt p) s -> p kt s", p=128)
            nc.sync.dma_start(out=xa0[:, 0:4, :], in_=xa_r[:, 0:4, 0:CH])
            nc.sync.dma_start(out=xa0[:, 4:8, :], in_=xa_r[:, 4:8, 0:CH])
            wq_sb = consts.tile([128, KT, DSL], F16, tag="wq")
            nc.sync.dma_start(out=wq_sb, in_=wq.rearrange("(kt p) m -> p kt m", p=128))
            xq0 = xqp.tile([128, KT, CH], F16, tag="xq", name="xq_c0")
            nc.sync.dma_start(
                out=xq0, in_=xq.rearrange("(kt p) s -> p kt s", p=128)[:, :, 0:CH])
            emb_sb = consts.tile([128, SKV], F16, tag="emb")
            nc.sync.dma_start(out=emb_sb, in_=emb2[:, :])
            wv_sb = consts.tile([128, KT, DSL], F16, tag="wv")
            nc.sync.dma_start(out=wv_sb, in_=wv.rearrange("(kt p) m -> p kt m", p=128))
            bq_sb = consts.tile([128, 2], F32, tag="bq")
            nc.sync.dma_start(out=bq_sb, in_=bq2[:, :])
            bk_sb = consts.tile([128, 2], F32, tag="bk")
            nc.sync.dma_start(out=bk_sb, in_=bk2[:, :])
            bv_sb = consts.tile([1, DSL], F32, tag="bv")
            nc.sync.dma_start(out=bv_sb, in_=bvr[:, :])
            wo_sb = consts.tile([128, 2, DIM], F16, tag="wo")
            nc.sync.dma_start(out=wo_sb, in_=wo.rearrange("(kt p) m -> p kt m", p=128))

            onescol_f = consts.tile([128, ST * HPC], F32, tag="onescol")
            nc.vector.memset(onescol_f, 1.0)
            # bv broadcast across partitions once; fused into the psv->v4 copy
            bv_bc = consts.tile([128, DSL], F32, tag="bv_bc")
            nc.gpsimd.partition_broadcast(bv_bc, bv_sb)

            work_q = []       # (slot_idx, unit) zipper queue, started in ph.1
            pushed = set()

            # ---- persistent activations ----
            qT = big.tile([128, 2, SQ], F16, tag="qT")
            kT = big.tile([128, 2, SKV], F16, tag="kT")
            oT0 = big.tile([128, SQ], F16, tag="oT0")
            oT1 = big.tile([128, SQ], F16, tag="oT1")
            oTs = [oT0, oT1]
            v4 = big.tile([128, ST, HPC, 68], F16, tag="v4")
            nc.vector.tensor_copy(
                v4[:, :, :, 64:65],
                onescol_f.rearrange("p (s g) -> p s g", s=ST).unsqueeze(3))

            # ---- scores+exp for a head PAIR (hp -> heads 2hp, 2hp+1) ----
            # The two heads' matmuls alternate T0 (partitions 0-63) / T8
            # (64-127) row-tiles so they execute concurrently on the PE.
            exps_store = {}
            next_pair = {}

            def _scores_pair(hp, c, p):
                # One psum tile per skv-tile s2 holds BOTH heads of the pair
                # side by side ([:, 0:CH] head 2hp via T0, [:, CH:] head 2hp+1
                # via T8) so the two row-tiled matmuls share one allocation
                # (no skewed waits) and one exp covers both heads.
                mt = hp
                for half in range(2):
                    s2 = 2 * p + half
                    pss = ps1024.tile([128, 2 * CH], F32, tag="sc",
                                      name=f"pss{hp}_{c}_{s2}")
                    for i in range(2):
                        pb = i * 64
                        nc.tensor.matmul(
                            pss[:, i * CH:(i + 1) * CH],
                            kT[pb:pb + 64, mt, s2 * 128:(s2 + 1) * 128],
                            qT[pb:pb + 64, mt, c * CH:(c + 1) * CH],
                            start=True, stop=True)
                    et = expp.tile([128, 2 * CH], F16, tag="exp",
                                   name=f"et{hp}_{c}_{s2}")
                    if s2 in DVE_EXP_S2:
                        # offload this tile's exp to the otherwise-idle DVE:
                        # Schraudolph bit-trick, i16 = scores*184.665+15326
                        # bitcast as f16 ~= exp(scores/8) (~1.5% rms, on 2/16
                        # tiles -> ~5e-3 on the final output)
                        nc.vector.tensor_scalar(
                            et.bitcast(mybir.dt.int16), pss,
                            SCALE * 1477.3197, 15326.0,
                            ALU.mult, ALU.add)
                    else:
                        nc.scalar.activation(et, pss, AF.Exp, scale=SCALE)
                    exps_store.setdefault((hp, c), []).append(et)
                next_pair[(hp, c)] = p + 1

            xq_tiles = {0: xq0}
            xa_tiles = {0: xa0}

            def _fetch_xq(c):
                if c not in xq_tiles and c < NCH:
                    xt = xqp.tile([128, KT, CH], F16, tag="xq", name=f"xq_c{c}")
                    nc.sync.dma_start(
                        out=xt,
                        in_=xq.rearrange("(kt p) s -> p kt s",
                                         p=128)[:, :, c * CH:(c + 1) * CH])
                    xq_tiles[c] = xt

            def _fetch_xa(c):
                if c not in xa_tiles and c < NCH:
                    xt = xap.tile([128, KT, CH], F16, tag="xa", name=f"xa_c{c}")
                    nc.sync.dma_start(
                        out=xt,
                        in_=xa_r[:, :, c * CH:(c + 1) * CH])
                    xa_tiles[c] = xt

            def _qproj(c):
                _fetch_xq(c)
                xt = xq_tiles[c]
                psq = [ps512.tile([128, CH], F32, tag="mm", name=f"psq{c}_{i}")
                       for i in range(2)]
                for kt in range(KT):
                    for mt in range(2):
                        nc.tensor.matmul(psq[mt], wq_sb[:, kt, mt * 128:(mt + 1) * 128],
                                         xt[:, kt, :], start=(kt == 0),
                                         stop=(kt == KT - 1))
                for mt in range(2):
                    nc.vector.tensor_scalar_add(qT[:, mt, c * CH:(c + 1) * CH],
                                                psq[mt], bq_sb[:, mt:mt + 1])

            # ---- phase 1: k/v/q projections; scores+exp for pair 0 of
            # chunks 0-1 are pre-scheduled so ACT works during this phase ----
            for c in range(NCH):
                _fetch_xa(c)
                xat = xa_tiles[c]
                psk = [ps512.tile([128, CH], F32, tag="mm", name=f"psk{c}_{i}")
                       for i in range(2)]
                for kt in range(KT):
                    for mt in range(2):
                        nc.tensor.matmul(psk[mt], wk_sb[:, kt, mt * 128:(mt + 1) * 128],
                                         xat[:, kt, :], start=(kt == 0),
                                         stop=(kt == KT - 1))
                # prefetch next chunk's activations behind this chunk's use
                _fetch_xa(c + 1)
                _fetch_xq(c + 1)
                for mt in range(2):
                    # kT = (psum + bk) + emb   (emb rows duplicated across both head halves)
                    nc.vector.scalar_tensor_tensor(
                        kT[:, mt, c * CH:(c + 1) * CH], psk[mt], bk_sb[:, mt:mt + 1],
                        emb_sb[:, c * CH:(c + 1) * CH], ALU.add, ALU.add)
                _qproj(c)
                # preschedule chunk-0 scores BEFORE the v-projection: pair 0
                # fully, pair 1 first half (24 of the 32 exp-pool tiles) so
                # ACT starts exping as early as possible; production stays in
                # slot order so pool reuse never outruns the not-yet-issued
                # AV readers
                for php, cap in ((0, ST // 2), (1, 4)):
                    for p in range(next_pair.get((php, 0), 0),
                                   min(cap, 2 * (c + 1))):
                        _scores_pair(php, 0, p)
                for j in range(HPC):
                    st = c * HPC + j
                    # v psums use the AV pool (idle during the kv phase) so
                    # early attention scores get the ps1024 slots
                    psv = psav.tile([128, CH], F32, tag="av")
                    for kt in range(KT):
                        nc.tensor.matmul(psv[:, 0:DSL],
                                         xat[:, kt, j * 128:(j + 1) * 128],
                                         wv_sb[:, kt, :], start=(kt == 0),
                                         stop=(kt == KT - 1))
                    # v4 = psv + bv (broadcast), fused into the eviction copy
                    nc.vector.scalar_tensor_tensor(
                        v4[:, st, :, 0:64],
                        psv[:, 0:DSL].rearrange("p (g m) -> p g m", g=HPC), 1.0,
                        bv_bc.rearrange("p (g m) -> p g m", g=HPC),
                        ALU.mult, ALU.add)
                # once pair (0,0) is fully scored, its AV can start filling
                # the remaining phase-1 PE slack
                if next_pair.get((0, 0), 0) == ST // 2 and 0 not in pushed:
                    pushed.add(0)
                    work_q.extend((0, u) for u in _av_units(0, 0))
                if c == NCH - 1:
                    for _ in range(min(10, len(work_q))):
                        work_q.pop(0)[1]()

            # ---- phase 2: per head-pair slot: scores+exp zippered with the
            # previous slot's AV+normalize via a work queue.  AV matmuls are
            # split into 64-contraction halves on row-tiles T0/T8 (two psum
            # accumulators, DVE-summed at the end) so the WHOLE attention
            # stretch stays in the 64x128 tiling mode: no PE mode-switch
            # drains, AV LDWEIGHTS hide under the other tile's matmul, and
            # consecutive T0/T8 streams run concurrently.  Out-proj runs as
            # per-chunk blocks at slot boundaries (2 mode switches each). ----
            def _av_units(hp, c):
                units = []
                for i in range(2):
                    h = 2 * hp + i
                    mt, pb = h // 2, (h % 2) * 64
                    state = {}

                    def mk_mm(h, c, state, s2a):
                        def go():
                            if "pavA" not in state:
                                state["pavA"] = psav.tile([128, CH], F32, tag="av",
                                                          name=f"pavA{h}_{c}")
                                if AV_SPLIT:
                                    state["pavB"] = psav.tile(
                                        [128, CH], F32, tag="av",
                                        name=f"pavB{h}_{c}")
                                state["exps"] = exps_store[(h // 2, c)]
                            pavA, exps = state["pavA"], state["exps"]
                            sl = slice((h % 2) * CH, (h % 2 + 1) * CH)
                            for s2 in (s2a, s2a + 1):
                                if AV_SPLIT:
                                    nc.tensor.matmul(
                                        pavA[0:65, :], v4[0:64, s2, h, 0:65],
                                        exps[s2][0:64, sl],
                                        start=(s2 == 0), stop=(s2 == ST - 1))
                                    nc.tensor.matmul(
                                        state["pavB"][0:65, :],
                                        v4[64:128, s2, h, 0:65],
                                        exps[s2][64:128, sl],
                                        start=(s2 == 0), stop=(s2 == ST - 1))
                                else:
                                    nc.tensor.matmul(
                                        pavA[0:65, :], v4[:, s2, h, 0:65],
                                        exps[s2][:, sl],
                                        start=(s2 == 0), stop=(s2 == ST - 1))
                        return go

                    for s2a in range(0, ST, 2):
                        units.append(mk_mm(h, c, state, s2a))

                    def mk_norm(h, mt, pb, state, c):
                        def go():
                            if AV_SPLIT:
                                # PSUM has one DVE read port: evacuate pavA,
                                # then fuse the pavB add (PSUM + SBUF in).
                                halfA = smallp.tile([65, CH], F32, tag="avhalf",
                                                    name=f"avh{h}_{c}")
                                nc.vector.tensor_copy(halfA,
                                                      state["pavA"][0:65, :])
                                sum_sb = smallp.tile([65, CH], F32, tag="avsum",
                                                     name=f"avs{h}_{c}")
                                nc.vector.scalar_tensor_tensor(
                                    sum_sb, state["pavB"][0:65, :], 1.0, halfA,
                                    ALU.mult, ALU.add)
                            else:
                                sum_sb = smallp.tile([65, CH], F32, tag="avsum",
                                                     name=f"avs{h}_{c}")
                                nc.vector.tensor_copy(sum_sb,
                                                      state["pavA"][0:65, :])
                            # the custom reciprocal op needs a partition-0
                            # input (base_partition 64 misbehaves), so stage
                            # the denominator row through a small copy
                            denrow = smallp.tile([1, CH], F32, tag="denrow",
                                                 name=f"den{h}_{c}")
                            nc.vector.tensor_copy(denrow, sum_sb[64:65, :])
                            drec = smallp.tile([1, CH], F32, tag="drec",
                                               name=f"drec{h}_{c}")
                            nc.vector.reciprocal_approx_fast(drec, denrow)
                            bc_sb = smallp.tile([64, CH], F32, tag="bcs",
                                                name=f"bcs{h}_{c}")
                            nc.gpsimd.partition_broadcast(bc_sb, drec)
                            nc.vector.tensor_mul(
                                oTs[mt][pb:pb + 64, c * CH:(c + 1) * CH],
                                sum_sb[0:64, :], bc_sb)
                        return go

                    units.append(mk_norm(h, mt, pb, state, c))
                return units

            def _outproj(c, alternate=False):
                for e in range(8):
                    pso = ps512.tile([128, CH], F32, tag="mm", name=f"pso{c}_{e}")
                    for kt in range(2):
                        nc.tensor.matmul(pso, wo_sb[:, kt, e * 128:(e + 1) * 128],
                                         oTs[kt][:, c * CH:(c + 1) * CH],
                                         start=(kt == 0), stop=(kt == 1))
                    # f16 eviction: 2x DVE rate and half the output DMA; the
                    # host sums the four per-core partials in fp32
                    ot_sb = evp.tile([128, CH], F16, tag="ev", name=f"ot{c}_{e}")
                    # final chunk: alternate DVE/ACT evictions so the tail
                    # isn't serialized on one engine (ACT is idle by then;
                    # mid-kernel it would head-of-line-block the exps)
                    if alternate and e % 2 == 1:
                        nc.scalar.copy(ot_sb, pso)
                    else:
                        nc.vector.tensor_copy(ot_sb, pso)
                    nc.sync.dma_start(out=out[e * 128:(e + 1) * 128, c * CH:(c + 1) * CH],
                                      in_=ot_sb)

            slots = [(hp, c) for c in range(NCH) for hp in range(2)]
            for si, (hp, c) in enumerate(slots):
                # hard guard: everything queued 2+ slots ago must be issued
                # now -- its exp tiles' pool buffers get reallocated by this
                # slot, and out-proj below needs the prior chunk's normalize.
                while work_q and work_q[0][0] <= si - 2:
                    work_q.pop(0)[1]()
                if hp == 1 and c >= 1:
                    _outproj(c - 1)
                for p in range(next_pair.get((hp, c), 0), ST // 2):
                    for _ in range(min(3, len(work_q))):
                        work_q.pop(0)[1]()
                    _scores_pair(hp, c, p)
                if si not in pushed:
                    pushed.add(si)
                    work_q.extend((si, u) for u in _av_units(hp, c))
            for _, u in work_q:
                u()
            _outproj(NCH - 1, alternate=True)

    nc.compile()
    return nc


def _make_runner(nc):
    """Build a reusable jitted SPMD executor (mirrors bass2jax.run_bass_via_pjrt)."""
    import jax
    import numpy as _np
    from jax.sharding import Mesh, PartitionSpec
    from jax.experimental.shard_map import shard_map
    import concourse.mybir as mybir
    from concourse.bass2jax import (_bass_exec_p, install_neuronx_cc_hook,
                                    partition_id_tensor)

    install_neuronx_cc_hook()
    partition_name = nc.partition_id_tensor.name if nc.partition_id_tensor else None

    in_names, out_names, out_avals, zero_outs = [], [], [], []
    for alloc in nc.m.functions[0].allocations:
        if not isinstance(alloc, mybir.MemoryLocationSet):
            continue
        name = alloc.memorylocations[0].name
        if alloc.kind == "ExternalInput":
            if name != partition_name:
                in_names.append(name)
        elif alloc.kind == "ExternalOutput":
            shape = tuple(alloc.tensor_shape)
            dtype = mybir.dt.np(alloc.dtype)
            out_names.append(name)
            out_avals.append(jax.core.ShapedArray(shape, dtype))
            zero_outs.append(_np.zeros(shape, dtype))
    n_params = len(in_names)
    n_outs = len(out_avals)
    all_in_names = list(in_names) + list(out_names)
    if partition_name is not None:
        all_in_names.append(partition_name)
    donate = tuple(range(n_params, n_params + n_outs))

    def _body(*args):
        operands = list(args)
        if partition_name is not None:
            operands.append(partition_id_tensor())
        outs = _bass_exec_p.bind(
            *operands,
            out_avals=tuple(out_avals),
            in_names=tuple(all_in_names),
            out_names=tuple(out_names),
            lowering_input_output_aliases=(),
            sim_require_finite=True,
            sim_require_nnan=True,
            nc=nc,
        )
        return tuple(outs)

    devices = jax.devices()[:N_CORES]
    mesh = Mesh(np.asarray(devices), ("core",))
    in_specs = (PartitionSpec("core"),) * (n_params + n_outs)
    out_specs = (PartitionSpec("core"),) * n_outs
    sharded = jax.jit(
        shard_map(_body, mesh=mesh, in_specs=in_specs, out_specs=out_specs,
                  check_rep=False),
        donate_argnums=donate, keep_unused=True)
    # non-donating variant for repeat-timing with device-resident operands
    sharded_nd = jax.jit(
        shard_map(_body, mesh=mesh, in_specs=in_specs, out_specs=out_specs,
                  check_rep=False),
        keep_unused=True)

    def _concat(in_maps):
        concat_in = [
            np.concatenate([np.asarray(in_maps[c][name]) for c in range(N_CORES)], axis=0)
            for name in in_names
        ]
        concat_zeros = [np.zeros((N_CORES * z.shape[0], *z.shape[1:]), z.dtype)
                        for z in zero_outs]
        return concat_in, concat_zeros

    def run(in_maps, unpack=True):
        concat_in, concat_zeros = _concat(in_maps)
        out_arrs = sharded(*concat_in, *concat_zeros)
        if not unpack:
            jax.block_until_ready(out_arrs)
            return None
        return [
            {name: np.asarray(out_arrs[i]).reshape(N_CORES, *out_avals[i].shape)[c]
             for i, name in enumerate(out_names)}
            for c in range(N_CORES)
        ]

    def stage(in_maps):
        """device_put all operands once; returns args for timed_call."""
        from jax.sharding import NamedSharding
        sh = NamedSharding(mesh, PartitionSpec("core"))
        concat_in, concat_zeros = _concat(in_maps)
        dev = [jax.device_put(x, sh) for x in concat_in + concat_zeros]
        jax.block_until_ready(dev)
        return dev

    def timed_call(dev_args):
        out_arrs = sharded_nd(*dev_args)
        jax.block_until_ready(out_arrs)
        return out_arrs

    run.stage = stage
    run.timed_call = timed_call
    return run


def _get_runner():
    if "runner" not in _CACHE:
        nc = _build()
        _CACHE["nc"] = nc
        _CACHE["runner"] = _make_runner(nc)
    return _CACHE["runner"]


def _lrope_embT(label_emb, labels):
    inv_freq = (1.0 / (10000.0 ** (np.arange(0, HEAD_DIM, 2, dtype=np.float32)
                                   / HEAD_DIM))).astype(np.float32)
    pos = np.arange(SKV, dtype=np.float32)
    freqs = np.outer(pos, inv_freq)
    emb = np.concatenate([np.sin(freqs), np.cos(freqs)], axis=-1).astype(np.float32)
    lab = np.asarray(label_emb, np.float32)[np.asarray(labels).astype(np.int64)]
    return emb, lab  # [SKV, HD], [B, HD]


def make_in_maps(visual_features, audio_features, audio_labels,
                 Wq, bq, Wk, bk, Wv, bv, Wo, bo, label_emb):
    vis = np.asarray(visual_features, np.float32)
    aud = np.asarray(audio_features, np.float32)
    Wq = np.asarray(Wq, np.float32)
    Wk = np.asarray(Wk, np.float32)
    Wv = np.asarray(Wv, np.float32)
    Wo = np.asarray(Wo, np.float32)
    bq = np.asarray(bq, np.float32)
    bk = np.asarray(bk, np.float32)
    bv = np.asarray(bv, np.float32)
    emb, lab = _lrope_embT(label_emb, audio_labels)

    xqs = [np.ascontiguousarray(vis[b].T).astype(np.float16) for b in range(B)]
    xas = [np.ascontiguousarray(aud[b].T).astype(np.float16) for b in range(B)]
    embs = []
    for b in range(B):
        embT = np.ascontiguousarray((emb * lab[b][None, :]).T)  # [64, SKV]
        embs.append(np.concatenate([embT, embT], axis=0).astype(np.float16))

    in_maps = []
    for core in range(N_CORES):
        b, g = core // HPC, core % HPC
        sl = slice(g * DSL, (g + 1) * DSL)
        in_maps.append({
            "xq": xqs[b],
            "xa": xas[b],
            "wq": np.ascontiguousarray(Wq[sl, :].T).astype(np.float16),
            "wk": np.ascontiguousarray(Wk[sl, :].T).astype(np.float16),
            "wv": np.ascontiguousarray(Wv[sl, :].T).astype(np.float16),
            "wo": np.ascontiguousarray(Wo[:, sl].T).astype(np.float16),
            "emb2": embs[b],
            "bq2": np.ascontiguousarray(bq[sl].reshape(2, 128).T),
            "bk2": np.ascontiguousarray(bk[sl].reshape(2, 128).T),
            "bvr": np.ascontiguousarray(bv[sl].reshape(1, DSL)),
        })
    return in_maps


def kernel(**inputs):
    run = _get_runner()
    in_maps = make_in_maps(**inputs)
    results = run(in_maps)
    bo = np.asarray(inputs["bo"], np.float32)
    out = np.empty((B, SQ, DIM), np.float32)
    for b in range(B):
        s = results[4 * b]["out"].astype(np.float32)
        for g in range(1, HPC):
            s = s + results[4 * b + g]["out"].astype(np.float32)
        out[b] = s.T + bo[None, :]
    return out
